# revision 13
# baseline (speedup 1.0000x reference)
"""DoubleStreamBlock (FLUX-style) on 8 TRN2 NeuronCores.

Tensor-parallel across heads/columns:
 - qkv & mlp1 column-sharded (3 heads / 1536 mlp cols per core),
   proj & mlp2 row-sharded with ReduceScatter of partials.
 - Full-D elementwise work (ln, modulate, gate, residual) token-sharded:
   core r owns rows [txt 32r:32r+32 ; img 128r:128r+128].
 - Activations flow transposed ([feature, token]); AllGather of xm^T
   feeds the column-sharded matmuls.
 - Attention per-head, scores built directly transposed (k^T stationary),
   exp on ScalarE (scores are O(10): no max subtraction needed), row sums
   via ones-matmul, normalization folded in after the attn matmul.
 - All matmuls float32r (full-rate fp32, ~1e-4 rel rounding).
 - Partition broadcasts via K=1 ones-matmuls into PSUM.
"""

import sys
import time
import contextlib

sys.path.insert(0, "/opt/trn_rl_repo")

import numpy as np

import concourse.bass as bass
import concourse.mybir as mybir
import concourse.tile as tile
import bass_rust
from concourse.masks import make_identity

F32 = mybir.dt.float32
F32R = mybir.dt.float32r
AF = mybir.ActivationFunctionType
OP = mybir.AluOpType

NCORES = 8
D = 3072
KD = 24            # D / 128
HL = 3             # heads per core
DH = 128
LI, LT, L = 1024, 256, 1280
ML = 1536          # mlp hidden per core
KML = 12           # ML / 128
TR, IR, RR = 32, 128, 160   # txt rows, img rows, rows per core
EPS = 1e-6
RG = [list(range(NCORES))]
SQD = float(1.0 / np.sqrt(DH))

QT_TILES = [(0, 512), (512, 512), (1024, 256)]      # (start, width)
SRANGES = [(0, 256, "txt"), (256, 512, "img"), (768, 512, "img")]


# ---------------------------------------------------------------------------
# walrus only accepts one semaphore wait per instruction; Tile can emit
# more (esp. its tail drain). Hoist excess waits onto same-engine NOPs.
def _split_fat_waits(nc, limit=1):
    n_new = 0
    for bb in nc.main_func.blocks:
        insts = bb.instructions
        out = []
        changed = False
        for ins in insts:
            si = ins.sync_info
            if si is not None and len(si.on_wait) > limit:
                waits = list(si.on_wait)
                head, rest = waits[:-limit], waits[-limit:]
                while head:
                    chunk, head = head[:limit], head[limit:]
                    nop = mybir.InstNoOp(name=f"I-wsplit-{n_new}", ins=[], outs=[])
                    n_new += 1
                    nop.engine = ins.engine
                    nop.sync_info = bass_rust.SyncInfo(on_wait=chunk, on_update=[])
                    out.append(nop)
                ins.sync_info = bass_rust.SyncInfo(
                    on_wait=rest, on_update=list(si.on_update))
                changed = True
            out.append(ins)
        if changed:
            insts[:] = out
    return n_new


# ---------------------------------------------------------------------------
def build_nc():
    nc = bass.Bass()

    def I(name, shape):
        return nc.declare_dram_parameter(name, shape, F32, isOutput=False)

    p = {}
    p["x_rows"] = I("x_rows", [RR, D])
    p["vec"] = I("vec", [D])
    p["pe_a"] = I("pe_a", [DH, L])
    p["pe_b"] = I("pe_b", [DH, L])
    p["ones_row"] = I("ones_row", [1, 128])
    p["ones_col"] = I("ones_col", [128, 1])
    for s in ("img", "txt"):
        p[f"mwA_{s}"] = I(f"mwA_{s}", [D, 768])
        p[f"mbA_{s}"] = I(f"mbA_{s}", [1, 768])
        p[f"mwB_{s}"] = I(f"mwB_{s}", [D, 1536])
        p[f"mbB_{s}"] = I(f"mbB_{s}", [1, 1536])
        p[f"qkvw_{s}"] = I(f"qkvw_{s}", [D, 9 * 128])
        p[f"qkvb_{s}"] = I(f"qkvb_{s}", [9 * 128])
        p[f"qn_{s}"] = I(f"qn_{s}", [DH])
        p[f"kn_{s}"] = I(f"kn_{s}", [DH])
        p[f"pw_{s}"] = I(f"pw_{s}", [HL * DH, D])
        p[f"pb_{s}"] = I(f"pb_{s}", [1, D])
        p[f"w1_{s}"] = I(f"w1_{s}", [D, ML])
        p[f"b1_{s}"] = I(f"b1_{s}", [ML])
        p[f"w2_{s}"] = I(f"w2_{s}", [ML, D])
        p[f"b2_{s}"] = I(f"b2_{s}", [1, D])

    out_rows = nc.declare_dram_parameter("out_rows", [RR, D], F32, isOutput=True)

    modA_b = nc.dram_tensor("modA_b", [2, 768], F32)
    modA_f = nc.dram_tensor("modA_f", [NCORES, 2, 768], F32, addr_space="Shared")
    modB_b = nc.dram_tensor("modB_b", [2, 1536], F32)
    modB_f = nc.dram_tensor("modB_f", [NCORES, 2, 1536], F32, addr_space="Shared")
    xm_b = nc.dram_tensor("xm_b", [D, RR], F32)
    xm_f = nc.dram_tensor("xm_f", [NCORES, D, RR], F32, addr_space="Shared")
    xm2_b = nc.dram_tensor("xm2_b", [D, RR], F32)
    xm2_f = nc.dram_tensor("xm2_f", [NCORES, D, RR], F32, addr_space="Shared")
    pp_b = nc.dram_tensor("pp_b", [NCORES, RR, D], F32)
    pp_r = nc.dram_tensor("pp_r", [RR, D], F32)
    mp_b = nc.dram_tensor("mp_b", [NCORES, RR, D], F32)
    mp_r = nc.dram_tensor("mp_r", [RR, D], F32)

    streams = ("txt", "img")
    ROWR = {"txt": (0, TR), "img": (TR, IR)}

    SWAP_MASK = []
    for i in range(16):
        SWAP_MASK += [2 * i + 1, 2 * i]

    with tile.TileContext(nc) as tc:
        top = contextlib.ExitStack()
        with top:
            const = top.enter_context(tc.tile_pool(name="const", bufs=1))
            persist = top.enter_context(tc.tile_pool(name="persist", bufs=1))

            ones_r = const.tile([1, 128], F32R)
            nc.sync.dma_start(ones_r[:], p["ones_row"].ap().bitcast(F32R))
            ones_c = const.tile([128, 1], F32R)
            nc.sync.dma_start(ones_c[:], p["ones_col"].ap().bitcast(F32R))
            ident = const.tile([128, 128], F32)
            make_identity(nc, ident[:])
            eps_t = const.tile([128, 1], F32)
            nc.vector.memset(eps_t[:], EPS)
            peA = const.tile([128, L], F32)
            peB = const.tile([128, L], F32)
            nc.sync.dma_start(peA[:], p["pe_a"].ap())
            nc.sync.dma_start(peB[:], p["pe_b"].ap())
            wn_t = {}
            for s in streams:
                for w in ("qn", "kn"):
                    t = const.tile([128, 1], F32, name=f"{w}_{s}")
                    nc.sync.dma_start(
                        t[:], p[f"{w}_{s}"].ap().rearrange("(d o) -> d o", o=1))
                    wn_t[(w, s)] = t
            qkvb_t, b1_t = {}, {}
            for s in streams:
                qkvb_t[s] = const.tile([128, 9], F32, name=f"qkvb_{s}")
                nc.sync.dma_start(
                    qkvb_t[s][:], p[f"qkvb_{s}"].ap().rearrange("(c q) -> q c", q=128))
                b1_t[s] = const.tile([128, KML], F32, name=f"b1_{s}")
                nc.sync.dma_start(
                    b1_t[s][:], p[f"b1_{s}"].ap().rearrange("(c q) -> q c", q=128))

            def bcast(psum_pool, row_ap, npart, width, tag):
                """[1, width] f32r row -> [npart, width] PSUM tile."""
                t = psum_pool.tile([npart, 512], F32, name=tag)[:, :width]
                nc.tensor.matmul(t, ones_r[:, :npart], row_ap,
                                 start=True, stop=True)
                return t

            def mod_vec_row(pool, mod_f, si, vslot, tag):
                """[1, D] f32r SBUF view of a gathered modulation vector."""
                t = pool.tile([1, NCORES, 384], F32R, name=tag)
                src = mod_f.ap()[:, si, 384 * vslot:384 * (vslot + 1)]
                nc.sync.dma_start(
                    t[:], src.rearrange("(o r) c -> o r c", o=1).bitcast(F32R))
                return t[:].rearrange("q r c -> q (r c)")

            def ln_stats(pool, x_ap, n, tag):
                st = pool.tile([n, 6, 6], F32, name=f"st{tag}")
                xr = x_ap.rearrange("q (g f) -> q g f", f=512)
                for g in range(6):
                    nc.vector.bn_stats(st[:, g, :], xr[:, g, :])
                mv = pool.tile([n, 2], F32, name=f"mv{tag}")
                nc.vector.bn_aggr(mv[:], st[:])
                nm = pool.tile([n, 1], F32, name=f"nm{tag}")
                nc.vector.tensor_scalar_mul(nm[:], mv[:, 0:1], -1.0)
                iv = pool.tile([n, 1], F32, name=f"iv{tag}")
                nc.scalar.activation(iv[:], mv[:, 1:2], AF.Sqrt, bias=eps_t[:n])
                nc.vector.reciprocal(iv[:], iv[:])
                return nm, iv

            def ln_mod_transpose(pool, psp, x_tiles, mod_f, vsh, vsc, bounce):
                """ln + modulate both streams' rows, transpose, DMA to bounce."""
                xmT = pool.tile([128, KD, RR], F32, name="xmT")
                for si, s in enumerate(streams):
                    off, n = ROWR[s]
                    xt = x_tiles[s]
                    nm, iv = ln_stats(pool, xt[:], n, s)
                    y = pool.tile([n, D], F32, name=f"y_{s}")
                    nc.vector.tensor_scalar(
                        y[:], xt[:], nm[:], iv[:], op0=OP.add, op1=OP.mult)
                    sh_row = mod_vec_row(pool, mod_f, si, vsh, "shrow")
                    sc_row = mod_vec_row(pool, mod_f, si, vsc, "scrow")
                    for dt in range(6):
                        sl = slice(512 * dt, 512 * (dt + 1))
                        scp = bcast(psp, sc_row[:, sl], n, 512, "bc")
                        shp = bcast(psp, sh_row[:, sl], n, 512, "bc")
                        xm = pool.tile([n, 512], F32, name="xmsl")
                        nc.vector.tensor_mul(xm[:], y[:, sl], scp)
                        nc.vector.tensor_add(xm[:], xm[:], shp)
                        for j in range(4):
                            k = 4 * dt + j
                            tp = psp.tile([128, 128], F32, name="tp")[:, :n]
                            nc.tensor.transpose(
                                tp, xm[:, 128 * j:128 * (j + 1)], ident[:n, :n])
                            nc.vector.tensor_copy(xmT[:, k, off:off + n], tp)
                nc.sync.dma_start(
                    bounce.ap().rearrange("(k q) r -> q k r", q=128), xmT[:])

            def xmT_rhs(xmf, tbuf, k, t0, tw):
                """DMA a [128, tw] f32r rhs tile (contraction chunk k,
                tokens t0:t0+tw) out of the AllGathered xm^T buffer."""
                if t0 < LT:
                    r0, nr = t0 // TR, tw // TR
                    src = xmf.ap()[r0:r0 + nr, 128 * k:128 * (k + 1), 0:TR]
                else:
                    i0 = t0 - LT
                    r0, nr = i0 // IR, tw // IR
                    src = xmf.ap()[r0:r0 + nr, 128 * k:128 * (k + 1), TR:RR]
                nc.sync.dma_start(
                    tbuf.rearrange("q (r c) -> q r c", r=nr),
                    src.rearrange("r q c -> q r c").bitcast(F32R))

            def part_out_dma(bounce, tc0, d0, src_tile):
                """[128, 512] partial tile (token chunk tc0) -> rank-blocked
                partial buffer rows."""
                t0 = 128 * tc0
                if t0 < LT:
                    for rr in range(4):
                        r = t0 // TR + rr
                        nc.sync.dma_start(
                            bounce.ap()[r, 0:TR, d0:d0 + 512],
                            src_tile[32 * rr:32 * (rr + 1), :])
                else:
                    r = (t0 - LT) // IR
                    nc.sync.dma_start(
                        bounce.ap()[r, TR:RR, d0:d0 + 512], src_tile)

            # =============================================================
            # P0: silu(vec)
            vec_t = const.tile([128, KD], F32)
            nc.sync.dma_start(vec_t[:], p["vec"].ap().rearrange("(k q) -> q k", q=128))
            vec_s = const.tile([128, KD], F32R)
            nc.scalar.activation(vec_s[:], vec_t[:], AF.Silu)

            # x rows (persist: needed for residual in P5)
            x_t = {}
            for s in streams:
                off, n = ROWR[s]
                x_t[s] = persist.tile([n, D], F32, name=f"x_{s}")
                nc.sync.dma_start(x_t[s][:], p["x_rows"].ap()[off:off + n, :])
            x2_t = {}
            for s in streams:
                off, n = ROWR[s]
                x2_t[s] = persist.tile([n, D], F32, name=f"x2_{s}")

            # =============================================================
            with tc.tile_pool(name="modw", bufs=3) as modp, \
                 tc.tile_pool(name="modsb", bufs=1) as modsb, \
                 tc.tile_pool(name="modps", bufs=2, space="PSUM") as modps:

                def matvec(wparam, n_out, out_slice):
                    wview = wparam.ap().rearrange("(ko ki) n -> ki ko n", ki=128)
                    for nt in range(n_out // 384):
                        mp = modps.tile([1, 384], F32, name="mvps")
                        for k in range(KD):
                            wt = modp.tile([128, 384], F32R, name="mv_w")
                            nc.sync.dma_start(
                                wt[:],
                                wview[:, k, 384 * nt:384 * (nt + 1)].bitcast(F32R))
                            nc.tensor.matmul(
                                mp[:], vec_s[:, k:k + 1], wt[:],
                                start=(k == 0), stop=(k == KD - 1))
                        nc.vector.tensor_copy(
                            out_slice[:, 384 * nt:384 * (nt + 1)], mp[:])

                # P1A: shift1/scale1
                gA = modsb.tile([1, 2, 768], F32, name="gA")
                for si, s in enumerate(streams):
                    matvec(p[f"mwA_{s}"], 768, gA[:, si, :])
                    bA = modp.tile([1, 768], F32, name="mbA")
                    nc.sync.dma_start(bA[:], p[f"mbA_{s}"].ap())
                    nc.vector.tensor_add(gA[:, si, :], gA[:, si, :], bA[:])
                    nc.vector.tensor_scalar_add(
                        gA[:, si, 384:768], gA[:, si, 384:768], 1.0)
                    nc.sync.dma_start(modA_b.ap()[si:si + 1, :], gA[0:1, si, :])
                nc.gpsimd.collective_compute(
                    "AllGather", OP.bypass, replica_groups=RG,
                    ins=[modA_b.ap().opt()], outs=[modA_f.ap().opt()])

                # P2: ln1 + modulate -> xm^T -> AllGather
                with tc.tile_pool(name="rw2", bufs=1) as rw2, \
                     tc.tile_pool(name="rps2", bufs=1, space="PSUM") as _rps2, \
                     tc.tile_pool(name="rps2b", bufs=3, space="PSUM") as rps2:
                    ln_mod_transpose(rw2, rps2, x_t, modA_f, 0, 1, xm_b)
                nc.gpsimd.collective_compute(
                    "AllGather", OP.bypass, replica_groups=RG,
                    ins=[xm_b.ap().opt()], outs=[xm_f.ap().opt()])

                # P1B: gate1/shift2/scale2/gate2 (traced after the critical
                # P2 chain so its DMAs don't get priority over it)
                gB = modsb.tile([1, 2, 1536], F32, name="gB")
                for si, s in enumerate(streams):
                    matvec(p[f"mwB_{s}"], 1536, gB[:, si, :])
                    bB = modp.tile([1, 1536], F32, name="mbB")
                    nc.sync.dma_start(bB[:], p[f"mbB_{s}"].ap())
                    nc.vector.tensor_add(gB[:, si, :], gB[:, si, :], bB[:])
                    nc.vector.tensor_scalar_add(
                        gB[:, si, 768:1152], gB[:, si, 768:1152], 1.0)
                    nc.sync.dma_start(modB_b.ap()[si:si + 1, :], gB[0:1, si, :])
                nc.gpsimd.collective_compute(
                    "AllGather", OP.bypass, replica_groups=RG,
                    ins=[modB_b.ap().opt()], outs=[modB_f.ap().opt()])

            # =============================================================
            # P3/P4: qkv, rope, attention, proj
            with tc.tile_pool(name="qk", bufs=1) as qk:
                qkT = qk.tile([128, 6, L], F32R, name="qkT")
                vT = qk.tile([128, 3, L], F32, name="vT")
                attnT = qk.tile([128, HL, L], F32R, name="attnT")

                with tc.tile_pool(name="qkw", bufs=3) as qkp, \
                     tc.tile_pool(name="qkrhs", bufs=1) as qkrhs, \
                     tc.tile_pool(name="qkwk", bufs=1) as qwk, \
                     tc.tile_pool(name="qke", bufs=2) as qke:

                    qkv_psum = tc.tile_pool(name="qkps", bufs=3, space="PSUM")
                    qkps = qkv_psum.__enter__()
                    STOK = {"txt": [(0, 256)], "img": [(256, 512), (768, 512)]}
                    for s in streams:
                        for (t0, tw) in STOK[s]:
                            rhs_all = qkrhs.tile([128, KD, 512], F32R,
                                                 name="qk_rhs")
                            for k in range(KD):
                                xmT_rhs(xm_f, rhs_all[:, k, :tw], k, t0, tw)
                            for cb in range(9):
                                mp = qkps.tile([128, 512], F32,
                                               name="qk_ps")[:, :tw]
                                for k in range(KD):
                                    wt = qkp.tile([128, 128], F32R, name="qk_w")
                                    nc.sync.dma_start(
                                        wt[:],
                                        p[f"qkvw_{s}"].ap()[
                                            128 * k:128 * (k + 1),
                                            128 * cb:128 * (cb + 1)].bitcast(F32R))
                                    nc.tensor.matmul(
                                        mp, wt[:], rhs_all[:, k, :tw],
                                        start=(k == 0), stop=(k == KD - 1))
                                dst = (qkT[:, cb, t0:t0 + tw] if cb < 6
                                       else vT[:, cb - 6, t0:t0 + tw])
                                nc.scalar.activation(
                                    dst, mp, AF.Identity,
                                    bias=qkvb_t[s][:, cb:cb + 1])

                    qkv_psum.__exit__(None, None, None)
                    rope_psum1 = tc.tile_pool(name="rops", bufs=1, space="PSUM")
                    rops = rope_psum1.__enter__()
                    rope_psum2 = tc.tile_pool(name="rops2", bufs=2, space="PSUM")
                    rops2 = rope_psum2.__enter__()
                    # rope + rms (in place on qkT)
                    for ci in range(6):
                        w = "qn" if ci < 3 else "kn"
                        src = qkT[:, ci, :]
                        sq = qwk.tile([128, L], F32R, name="rsq")
                        nc.vector.tensor_mul(sq[:], src, src)
                        ssp = rops.tile([1, L], F32, name="ssps")
                        for (t0, tw) in QT_TILES:
                            nc.tensor.matmul(
                                ssp[:, t0:t0 + tw], ones_c[:], sq[:, t0:t0 + tw],
                                start=True, stop=True)
                        inv = qwk.tile([1, L], F32R, name="rinv")
                        with nc.allow_low_precision(reason="f32r is full width"):
                            nc.scalar.activation(inv[:], ssp[:], AF.Sqrt,
                                                 bias=eps_t[:1], scale=1.0 / DH)
                            nc.vector.reciprocal(inv[:], inv[:])
                        xs = qwk.tile([128, L], F32, name="rxs")
                        nc.vector.stream_shuffle(xs[:], src, SWAP_MASK)
                        m1 = qwk.tile([128, L], F32, name="rm1")
                        nc.vector.tensor_mul(m1[:], src, peA[:])
                        nc.vector.tensor_mul(xs[:], xs[:], peB[:])
                        nc.vector.tensor_add(m1[:], m1[:], xs[:])
                        for (t0, tw, s) in SRANGES:
                            ip = bcast(rops2, inv[:, t0:t0 + tw], 128, tw, "rip")
                            nc.vector.scalar_tensor_tensor(
                                qkT[:, ci, t0:t0 + tw], m1[:, t0:t0 + tw],
                                wn_t[(w, s)][:], ip, op0=OP.mult, op1=OP.mult)

                    rope_psum2.__exit__(None, None, None)
                    rope_psum1.__exit__(None, None, None)
                    # attention per head
                    with tc.tile_pool(name="atps", bufs=2, space="PSUM") as atps, \
                         tc.tile_pool(name="scps", bufs=3, space="PSUM") as scps:
                        for h in range(HL):
                            vn = qwk.tile([128, 10, 128], F32R, name="vn")
                            for c in range(10):
                                tp = scps.tile([128, 128], F32, name="sc")
                                nc.tensor.transpose(
                                    tp[:], vT[:, h, 128 * c:128 * (c + 1)],
                                    ident[:])
                                nc.vector.tensor_copy(vn[:, c, :], tp[:])
                            for (t0, tw) in QT_TILES:
                                sume = atps.tile([1, 512], F32,
                                                 name="sume")[:, :tw]
                                acc = atps.tile([128, 512], F32,
                                                name="acc")[:, :tw]
                                for c in range(10):
                                    sc = scps.tile([128, 512], F32,
                                                   name="sc")[:, :tw]
                                    nc.tensor.matmul(
                                        sc, qkT[:, 3 + h, 128 * c:128 * (c + 1)],
                                        qkT[:, h, t0:t0 + tw],
                                        start=True, stop=True)
                                    et = qke.tile([128, 512], F32R,
                                                  name="expt")[:, :tw]
                                    nc.scalar.activation(et, sc, AF.Exp,
                                                         scale=SQD)
                                    nc.tensor.matmul(sume, ones_c[:], et,
                                                     start=(c == 0),
                                                     stop=(c == 9))
                                    nc.tensor.matmul(acc, vn[:, c, :], et,
                                                     start=(c == 0),
                                                     stop=(c == 9))
                                rec = qke.tile([1, 512], F32R,
                                               name="rec")[:, :tw]
                                with nc.allow_low_precision(reason="f32r"):
                                    nc.vector.reciprocal(rec, sume)
                                rp = bcast(scps, rec, 128, tw, "sc")
                                acc_sb = qke.tile([128, 512], F32,
                                                  name="accsb")[:, :tw]
                                nc.scalar.copy(acc_sb, acc)
                                nc.vector.tensor_mul(
                                    attnT[:, h, t0:t0 + tw], acc_sb, rp)

                # proj partials
                with tc.tile_pool(name="prj", bufs=3) as prj, \
                     tc.tile_pool(name="prps", bufs=5, space="PSUM") as prps:
                    TGROUPS = [[0, 1], [2, 3, 4, 5, 6], [7, 8, 9]]
                    for tg in TGROUPS:
                        s = "txt" if tg[0] < 2 else "img"
                        for d in range(6):
                            pst = {}
                            for t in tg:
                                pst[t] = prps.tile([128, 512], F32, name="pt")
                            for hh in range(HL):
                                rhs = prj.tile([128, 512], F32R, name="pw")
                                nc.sync.dma_start(
                                    rhs[:],
                                    p[f"pw_{s}"].ap()[
                                        128 * hh:128 * (hh + 1),
                                        512 * d:512 * (d + 1)].bitcast(F32R))
                                for t in tg:
                                    nc.tensor.matmul(
                                        pst[t],
                                        attnT[:, hh, 128 * t:128 * (t + 1)],
                                        rhs[:], start=(hh == 0),
                                        stop=(hh == HL - 1))
                            for t in tg:
                                ev = prj.tile([128, 512], F32, name="pev")
                                nc.scalar.copy(ev[:], pst[t])
                                part_out_dma(pp_b, t, 512 * d, ev[:])
            nc.gpsimd.collective_compute(
                "ReduceScatter", OP.add, replica_groups=RG,
                ins=[pp_b.ap().opt()], outs=[pp_r.ap().opt()])

            # =============================================================
            # P5: residual + gate; ln2 + modulate -> xm2^T -> AllGather
            with tc.tile_pool(name="rw5", bufs=1) as rw5, \
                 tc.tile_pool(name="rps5", bufs=1, space="PSUM") as _r5, \
                 tc.tile_pool(name="rps5b", bufs=3, space="PSUM") as rps5:
                for si, s in enumerate(streams):
                    off, n = ROWR[s]
                    prt = rw5.tile([n, D], F32, name=f"pr_{s}")
                    nc.sync.dma_start(prt[:], pp_r.ap()[off:off + n, :])
                    pbr = rw5.tile([1, D], F32R, name="pbr")
                    nc.sync.dma_start(pbr[:], p[f"pb_{s}"].ap().bitcast(F32R))
                    g1r = mod_vec_row(rw5, modB_f, si, 0, "g1row")
                    x2 = x2_t[s]
                    for dt in range(6):
                        sl = slice(512 * dt, 512 * (dt + 1))
                        pbp = bcast(rps5, pbr[:, sl], n, 512, "bc")
                        g1p = bcast(rps5, g1r[:, sl], n, 512, "bc")
                        nc.vector.tensor_add(x2[:, sl], prt[:, sl], pbp)
                        nc.vector.tensor_mul(x2[:, sl], x2[:, sl], g1p)
                        nc.vector.tensor_add(x2[:, sl], x2[:, sl], x_t[s][:, sl])
                ln_mod_transpose(rw5, rps5, x2_t, modB_f, 1, 2, xm2_b)
            nc.gpsimd.collective_compute(
                "AllGather", OP.bypass, replica_groups=RG,
                ins=[xm2_b.ap().opt()], outs=[xm2_f.ap().opt()])

            # =============================================================
            # P6/P7: mlp per token-half
            HALF = [
                (0, [(0, 256, "txt"), (256, 384, "img")], [0, 1, 2, 3, 4]),
                (640, [(640, 320, "img"), (960, 320, "img")], [5, 6, 7, 8, 9]),
            ]
            with tc.tile_pool(name="mlrhs", bufs=1) as mlrhs, \
                 tc.tile_pool(name="mlw", bufs=3) as mlw:
                for (h0, ranges, tchunks) in HALF:
                    rhs_half = mlrhs.tile([128, KD, 640], F32R, name="ml_rhs")
                    for k in range(KD):
                        for t in tchunks:
                            o = 128 * t - h0
                            xmT_rhs(xm2_f, rhs_half[:, k, o:o + 128],
                                    k, 128 * t, 128)
                    hid = mlrhs.tile([128, KML, 640], F32R, name="ml_hid")
                    with tc.tile_pool(name="mlps", bufs=3,
                                      space="PSUM") as mlps:
                        for hc in range(KML):
                            pst = {}
                            for (t0, tw, s) in ranges:
                                pst[t0] = mlps.tile([128, 512], F32,
                                                    name="mlp")[:, :tw]
                            for k in range(KD):
                                for (t0, tw, s) in ranges:
                                    wt = mlw.tile([128, 128], F32R, name="w1")
                                    nc.sync.dma_start(
                                        wt[:],
                                        p[f"w1_{s}"].ap()[
                                            128 * k:128 * (k + 1),
                                            128 * hc:128 * (hc + 1)]
                                        .bitcast(F32R))
                                    nc.tensor.matmul(
                                        pst[t0], wt[:],
                                        rhs_half[:, k, t0 - h0:t0 - h0 + tw],
                                        start=(k == 0), stop=(k == KD - 1))
                            for (t0, tw, s) in ranges:
                                nc.scalar.activation(
                                    hid[:, hc, t0 - h0:t0 - h0 + tw], pst[t0],
                                    AF.Gelu_apprx_tanh,
                                    bias=b1_t[s][:, hc:hc + 1])
                    with tc.tile_pool(name="m2ps", bufs=5,
                                      space="PSUM") as m2ps:
                        for d in range(6):
                            pst2 = {}
                            for t in tchunks:
                                pst2[t] = m2ps.tile([128, 512], F32, name="m2")
                            for hc in range(KML):
                                w2t = {}
                                for s in set(x[2] for x in ranges):
                                    w2t[s] = mlw.tile([128, 512], F32R,
                                                      name="w2")
                                    nc.sync.dma_start(
                                        w2t[s][:],
                                        p[f"w2_{s}"].ap()[
                                            128 * hc:128 * (hc + 1),
                                            512 * d:512 * (d + 1)]
                                        .bitcast(F32R))
                                for t in tchunks:
                                    s = "txt" if t < 2 else "img"
                                    nc.tensor.matmul(
                                        pst2[t],
                                        hid[:, hc,
                                            128 * t - h0:128 * (t + 1) - h0],
                                        w2t[s][:], start=(hc == 0),
                                        stop=(hc == KML - 1))
                            for t in tchunks:
                                ev = mlw.tile([128, 512], F32, name="m2ev")
                                nc.scalar.copy(ev[:], pst2[t])
                                part_out_dma(mp_b, t, 512 * d, ev[:])
            nc.gpsimd.collective_compute(
                "ReduceScatter", OP.add, replica_groups=RG,
                ins=[mp_b.ap().opt()], outs=[mp_r.ap().opt()])

            # =============================================================
            # P8: final residual + gate -> out_rows
            with tc.tile_pool(name="rw8", bufs=1) as rw8, \
                 tc.tile_pool(name="rps8", bufs=3, space="PSUM") as rps8:
                for si, s in enumerate(streams):
                    off, n = ROWR[s]
                    mrt = rw8.tile([n, D], F32, name=f"mr_{s}")
                    nc.sync.dma_start(mrt[:], mp_r.ap()[off:off + n, :])
                    b2r = rw8.tile([1, D], F32R, name="b2r")
                    nc.sync.dma_start(b2r[:], p[f"b2_{s}"].ap().bitcast(F32R))
                    g2r = mod_vec_row(rw8, modB_f, si, 3, "g2row")
                    fin = rw8.tile([n, D], F32, name=f"fin_{s}")
                    for dt in range(6):
                        sl = slice(512 * dt, 512 * (dt + 1))
                        b2p = bcast(rps8, b2r[:, sl], n, 512, "bc")
                        g2p = bcast(rps8, g2r[:, sl], n, 512, "bc")
                        nc.vector.tensor_add(fin[:, sl], mrt[:, sl], b2p)
                        nc.vector.tensor_mul(fin[:, sl], fin[:, sl], g2p)
                        nc.vector.tensor_add(fin[:, sl], fin[:, sl],
                                             x2_t[s][:, sl])
                    nc.sync.dma_start(out_rows.ap()[off:off + n, :], fin[:])

    _split_fat_waits(nc)
    return nc


# ---------------------------------------------------------------------------
# host-side sharding / execution
_CACHE = {}


def _host_shards(inputs):
    f = lambda k: np.ascontiguousarray(np.asarray(inputs[k], np.float32))
    img, txt, vec, pe = f("img")[0], f("txt")[0], f("vec")[0], f("pe")
    pe4 = pe[0, 0]  # [L, 64, 2, 2]
    A = np.empty((L, DH), np.float32)
    B = np.empty((L, DH), np.float32)
    A[:, 0::2] = pe4[:, :, 0, 0]; A[:, 1::2] = pe4[:, :, 1, 1]
    B[:, 0::2] = pe4[:, :, 0, 1]; B[:, 1::2] = pe4[:, :, 1, 0]
    pe_a = np.ascontiguousarray(A.T)
    pe_b = np.ascontiguousarray(B.T)

    maps = []
    for r in range(NCORES):
        m = {
            "vec": vec, "pe_a": pe_a, "pe_b": pe_b,
            "ones_row": np.ones((1, 128), np.float32),
            "ones_col": np.ones((128, 1), np.float32),
            "x_rows": np.concatenate(
                [txt[TR * r:TR * (r + 1)], img[IR * r:IR * (r + 1)]], 0),
        }
        for s in ("img", "txt"):
            mw, mb = f(f"{s}_mod_w"), f(f"{s}_mod_b")
            cA = np.concatenate([np.arange(D * v + 384 * r, D * v + 384 * (r + 1))
                                 for v in (0, 1)])
            cB = np.concatenate([np.arange(D * v + 384 * r, D * v + 384 * (r + 1))
                                 for v in (2, 3, 4, 5)])
            m[f"mwA_{s}"] = np.ascontiguousarray(mw[:, cA])
            m[f"mbA_{s}"] = np.ascontiguousarray(mb[cA][None])
            m[f"mwB_{s}"] = np.ascontiguousarray(mw[:, cB])
            m[f"mbB_{s}"] = np.ascontiguousarray(mb[cB][None])
            qw, qb = f(f"{s}_qkv_w"), f(f"{s}_qkv_b")
            hs = [HL * r + j for j in range(HL)]
            cols = np.concatenate(
                [np.arange(D * blk + DH * h, D * blk + DH * (h + 1))
                 for blk in range(3) for h in hs])
            m[f"qkvw_{s}"] = np.ascontiguousarray(qw[:, cols])
            m[f"qkvb_{s}"] = np.ascontiguousarray(qb[cols])
            m[f"qn_{s}"] = f(f"{s}_qnorm")
            m[f"kn_{s}"] = f(f"{s}_knorm")
            rowsel = np.concatenate(
                [np.arange(DH * h, DH * (h + 1)) for h in hs])
            m[f"pw_{s}"] = np.ascontiguousarray(f(f"{s}_proj_w")[rowsel])
            m[f"pb_{s}"] = f(f"{s}_proj_b")[None]
            m[f"w1_{s}"] = np.ascontiguousarray(
                f(f"{s}_mlp_w1")[:, ML * r:ML * (r + 1)])
            m[f"b1_{s}"] = np.ascontiguousarray(
                f(f"{s}_mlp_b1")[ML * r:ML * (r + 1)])
            m[f"w2_{s}"] = np.ascontiguousarray(
                f(f"{s}_mlp_w2")[ML * r:ML * (r + 1)])
            m[f"b2_{s}"] = f(f"{s}_mlp_b2")[None]
        maps.append(m)
    return maps


def _build_exec(nc, n_cores):
    import jax
    from jax.sharding import Mesh, PartitionSpec
    from jax.experimental.shard_map import shard_map
    from concourse.bass2jax import (
        _bass_exec_p, install_neuronx_cc_hook, partition_id_tensor)

    install_neuronx_cc_hook()
    partition_name = (nc.partition_id_tensor.name
                      if nc.partition_id_tensor else None)
    in_names, out_names, out_avals, zero_outs = [], [], [], []
    for alloc in nc.m.functions[0].allocations:
        if not isinstance(alloc, mybir.MemoryLocationSet):
            continue
        name = alloc.memorylocations[0].name
        if alloc.kind == "ExternalInput":
            if name != partition_name:
                in_names.append(name)
        elif alloc.kind == "ExternalOutput":
            shape = tuple(alloc.tensor_shape)
            dtype = mybir.dt.np(alloc.dtype)
            out_names.append(name)
            out_avals.append(jax.core.ShapedArray(shape, dtype))
            zero_outs.append(np.zeros(shape, dtype))
    n_params, n_outs = len(in_names), len(out_avals)
    all_in = list(in_names) + list(out_names)
    if partition_name is not None:
        all_in.append(partition_name)

    def _body(*args):
        operands = list(args)
        if partition_name is not None:
            operands.append(partition_id_tensor())
        return tuple(_bass_exec_p.bind(
            *operands, out_avals=tuple(out_avals), in_names=tuple(all_in),
            out_names=tuple(out_names), lowering_input_output_aliases=(),
            sim_require_finite=True, sim_require_nnan=True, nc=nc))

    devices = jax.devices()[:n_cores]
    mesh = Mesh(np.asarray(devices), ("core",))
    donate = tuple(range(n_params, n_params + n_outs))
    sharded = jax.jit(
        shard_map(_body, mesh=mesh,
                  in_specs=(PartitionSpec("core"),) * (n_params + n_outs),
                  out_specs=(PartitionSpec("core"),) * n_outs,
                  check_rep=False),
        donate_argnums=donate, keep_unused=True)
    return sharded, in_names, out_names, out_avals, zero_outs, mesh


def _get_compiled():
    if "exec" not in _CACHE:
        nc = build_nc()
        _CACHE["exec"] = _build_exec(nc, NCORES)
    return _CACHE["exec"]


def run_sharded(in_maps, n_timing=0):
    import jax
    from jax.sharding import PartitionSpec, NamedSharding
    sharded, in_names, out_names, out_avals, zero_outs, mesh = _get_compiled()
    sharding = NamedSharding(mesh, PartitionSpec("core"))
    concat_in = []
    for k in in_names:
        a0 = np.asarray(in_maps[0][k])
        cat = np.concatenate([np.asarray(in_maps[c][k]) for c in range(NCORES)], 0)
        concat_in.append(np.ascontiguousarray(cat))
    dev_in = [jax.device_put(a, sharding) for a in concat_in]
    for a in dev_in:
        a.block_until_ready()

    def zeros():
        return [jax.device_put(
            np.zeros((NCORES * z.shape[0], *z.shape[1:]), z.dtype), sharding)
            for z in zero_outs]

    outs = sharded(*dev_in, *zeros())
    jax.block_until_ready(outs)
    results = [
        {name: np.asarray(outs[i]).reshape(NCORES, *out_avals[i].shape)[c]
         for i, name in enumerate(out_names)}
        for c in range(NCORES)]
    times = []
    for _ in range(n_timing):
        zs = zeros()
        jax.block_until_ready(zs)
        t0 = time.perf_counter()
        o = sharded(*dev_in, *zs)
        jax.block_until_ready(o)
        times.append(time.perf_counter() - t0)
    return results, times


def kernel(**inputs):
    maps = _host_shards(inputs)
    results, _ = run_sharded(maps, n_timing=0)
    img = np.empty((1, LI, D), np.float32)
    txt = np.empty((1, LT, D), np.float32)
    for r in range(NCORES):
        rows = results[r]["out_rows"]
        txt[0, TR * r:TR * (r + 1)] = rows[:TR]
        img[0, IR * r:IR * (r + 1)] = rows[TR:]
    return img, txt


# revision 20
# speedup vs baseline: 2.7983x; 2.7983x over previous
"""DoubleStreamBlock (FLUX-style) on 8 TRN2 NeuronCores.

Tensor-parallel across heads/columns:
 - qkv & mlp1 column-sharded (3 heads / 1536 mlp cols per core),
   proj & mlp2 row-sharded with ReduceScatter of partials.
 - Full-D elementwise work (ln, modulate, gate, residual) token-sharded:
   core r owns rows [txt 32r:32r+32 ; img 128r:128r+128].
 - Activations flow transposed ([feature, token]); AllGather of xm^T
   feeds the column-sharded matmuls.
 - Attention per-head, scores built directly transposed (k^T stationary),
   exp on ScalarE (scores are O(10): no max subtraction needed), row sums
   via ones-matmul, normalization folded in after the attn matmul.
 - All matmuls float32r (full-rate fp32, ~1e-4 rel rounding).
 - Partition broadcasts via K=1 ones-matmuls into PSUM.
"""

import sys
import time
import contextlib

sys.path.insert(0, "/opt/trn_rl_repo")

import numpy as np

import concourse.bass as bass
import concourse.mybir as mybir
import concourse.tile as tile
import bass_rust
from concourse.masks import make_identity

F32 = mybir.dt.float32
F32R = mybir.dt.float32r
AF = mybir.ActivationFunctionType
OP = mybir.AluOpType

NCORES = 8
D = 3072
KD = 24            # D / 128
HL = 3             # heads per core
DH = 128
LI, LT, L = 1024, 256, 1280
ML = 1536          # mlp hidden per core
KML = 12           # ML / 128
TR, IR, RR = 32, 128, 160   # txt rows, img rows, rows per core
EPS = 1e-6
RG = [list(range(NCORES))]
SQD = float(1.0 / np.sqrt(DH))

QT_TILES = [(0, 512), (512, 512), (1024, 256)]      # (start, width)
SRANGES = [(0, 256, "txt"), (256, 512, "img"), (768, 512, "img")]


# ---------------------------------------------------------------------------
# walrus only accepts one semaphore wait per instruction; Tile can emit
# more (esp. its tail drain). Hoist excess waits onto same-engine NOPs.
def _split_fat_waits(nc, limit=1):
    n_new = 0
    for bb in nc.main_func.blocks:
        insts = bb.instructions
        out = []
        changed = False
        for ins in insts:
            si = ins.sync_info
            if si is not None and len(si.on_wait) > limit:
                waits = list(si.on_wait)
                head, rest = waits[:-limit], waits[-limit:]
                while head:
                    chunk, head = head[:limit], head[limit:]
                    nop = mybir.InstNoOp(name=f"I-wsplit-{n_new}", ins=[], outs=[])
                    n_new += 1
                    nop.engine = ins.engine
                    nop.sync_info = bass_rust.SyncInfo(on_wait=chunk, on_update=[])
                    out.append(nop)
                ins.sync_info = bass_rust.SyncInfo(
                    on_wait=rest, on_update=list(si.on_update))
                changed = True
            out.append(ins)
        if changed:
            insts[:] = out
    return n_new


# ---------------------------------------------------------------------------
def build_nc():
    nc = bass.Bass()

    def I(name, shape):
        return nc.declare_dram_parameter(name, shape, F32, isOutput=False)

    p = {}
    p["x_rows"] = I("x_rows", [RR, D])
    p["vec"] = I("vec", [D])
    p["pe_a"] = I("pe_a", [DH, L])
    p["pe_b"] = I("pe_b", [DH, L])
    p["ones_row"] = I("ones_row", [1, 128])
    p["ones_col"] = I("ones_col", [128, 1])
    for s in ("img", "txt"):
        p[f"mwA_{s}"] = I(f"mwA_{s}", [D, 768])
        p[f"mbA_{s}"] = I(f"mbA_{s}", [1, 768])
        p[f"mwB_{s}"] = I(f"mwB_{s}", [D, 1536])
        p[f"mbB_{s}"] = I(f"mbB_{s}", [1, 1536])
        p[f"qkvw_{s}"] = I(f"qkvw_{s}", [D, 9 * 128])
        p[f"qkvb_{s}"] = I(f"qkvb_{s}", [9 * 128])
        p[f"qn_{s}"] = I(f"qn_{s}", [DH])
        p[f"kn_{s}"] = I(f"kn_{s}", [DH])
        p[f"pw_{s}"] = I(f"pw_{s}", [HL * DH, D])
        p[f"pb_{s}"] = I(f"pb_{s}", [1, D])
        p[f"w1_{s}"] = I(f"w1_{s}", [D, ML])
        p[f"b1_{s}"] = I(f"b1_{s}", [ML])
        p[f"w2_{s}"] = I(f"w2_{s}", [ML, D])
        p[f"b2_{s}"] = I(f"b2_{s}", [1, D])

    out_rows = nc.declare_dram_parameter("out_rows", [RR, D], F32, isOutput=True)

    modA_b = nc.dram_tensor("modA_b", [2, 768], F32)
    modA_f = nc.dram_tensor("modA_f", [NCORES, 2, 768], F32, addr_space="Shared")
    modB_b = nc.dram_tensor("modB_b", [2, 1536], F32)
    modB_f = nc.dram_tensor("modB_f", [NCORES, 2, 1536], F32, addr_space="Shared")
    xm_b = nc.dram_tensor("xm_b", [D, RR], F32)
    xm_f = nc.dram_tensor("xm_f", [NCORES, D, RR], F32, addr_space="Shared")
    xm2_b = nc.dram_tensor("xm2_b", [D, RR], F32)
    xm2_f = nc.dram_tensor("xm2_f", [NCORES, D, RR], F32, addr_space="Shared")
    pp_b = nc.dram_tensor("pp_b", [NCORES, RR, D], F32)
    pp_r = nc.dram_tensor("pp_r", [RR, D], F32)
    mp_b = nc.dram_tensor("mp_b", [NCORES, RR, D], F32)
    mp_r = nc.dram_tensor("mp_r", [RR, D], F32)

    streams = ("txt", "img")
    ROWR = {"txt": (0, TR), "img": (TR, IR)}

    SWAP_MASK = []
    for i in range(16):
        SWAP_MASK += [2 * i + 1, 2 * i]

    with tile.TileContext(nc) as tc:
        top = contextlib.ExitStack()
        with top:
            const = top.enter_context(tc.tile_pool(name="const", bufs=1))
            persist = top.enter_context(tc.tile_pool(name="persist", bufs=1))
            xin_pool = tc.tile_pool(name="xin", bufs=1)
            xin = xin_pool.__enter__()

            ones_r = const.tile([1, 128], F32R)
            nc.sync.dma_start(ones_r[:], p["ones_row"].ap().bitcast(F32R))
            ones_c = const.tile([128, 1], F32R)
            nc.sync.dma_start(ones_c[:], p["ones_col"].ap().bitcast(F32R))
            ident = const.tile([128, 128], F32)
            make_identity(nc, ident[:])
            eps_t = const.tile([128, 1], F32)
            nc.vector.memset(eps_t[:], EPS)
            peA = const.tile([128, L], F32)
            peB = const.tile([128, L], F32)
            nc.sync.dma_start(peA[:], p["pe_a"].ap())
            nc.sync.dma_start(peB[:], p["pe_b"].ap())
            wn_t = {}
            for s in streams:
                for w in ("qn", "kn"):
                    t = const.tile([128, 1], F32, name=f"{w}_{s}")
                    nc.sync.dma_start(
                        t[:], p[f"{w}_{s}"].ap().rearrange("(d o) -> d o", o=1))
                    wn_t[(w, s)] = t
            qkvb_t, b1_t = {}, {}
            for s in streams:
                qkvb_t[s] = const.tile([128, 9], F32, name=f"qkvb_{s}")
                nc.sync.dma_start(
                    qkvb_t[s][:], p[f"qkvb_{s}"].ap().rearrange("(c q) -> q c", q=128))
                b1_t[s] = const.tile([128, KML], F32, name=f"b1_{s}")
                nc.sync.dma_start(
                    b1_t[s][:], p[f"b1_{s}"].ap().rearrange("(c q) -> q c", q=128))

            def bcast(psum_pool, row_ap, npart, width, tag):
                """[1, width] f32r row -> [npart, width] PSUM tile."""
                t = psum_pool.tile([npart, 512], F32, name=tag)[:, :width]
                nc.tensor.matmul(t, ones_r[:, :npart], row_ap,
                                 start=True, stop=True)
                return t

            def mod_vec_row(pool, mod_f, si, vslot, tag):
                """[1, D] f32r SBUF view of a gathered modulation vector."""
                t = pool.tile([1, NCORES, 384], F32R, name=tag)
                src = mod_f.ap()[:, si, 384 * vslot:384 * (vslot + 1)]
                nc.sync.dma_start(
                    t[:], src.rearrange("(o r) c -> o r c", o=1).bitcast(F32R))
                return t[:].rearrange("q r c -> q (r c)")

            def ln_stats(pool, x_ap, n, tag):
                st = pool.tile([n, 6, 6], F32, name=f"st{tag}")
                xr = x_ap.rearrange("q (g f) -> q g f", f=512)
                for g in range(6):
                    nc.vector.bn_stats(st[:, g, :], xr[:, g, :])
                mv = pool.tile([n, 2], F32, name=f"mv{tag}")
                nc.vector.bn_aggr(mv[:], st[:])
                nm = pool.tile([n, 1], F32, name=f"nm{tag}")
                nc.vector.tensor_scalar_mul(nm[:], mv[:, 0:1], -1.0)
                iv = pool.tile([n, 1], F32, name=f"iv{tag}")
                nc.scalar.activation(iv[:], mv[:, 1:2], AF.Sqrt, bias=eps_t[:n])
                nc.vector.reciprocal(iv[:], iv[:])
                return nm, iv

            def ln_mod_transpose(pool, psp, x_tiles, mod_f, vsh, vsc, bounce):
                """ln + modulate both streams' rows, transpose, DMA to bounce."""
                xmT = pool.tile([128, KD, RR], F32, name="xmT")
                for si, s in enumerate(streams):
                    off, n = ROWR[s]
                    xt = x_tiles[s]
                    nm, iv = ln_stats(pool, xt[:], n, s)
                    y = pool.tile([n, D], F32, name=f"y_{s}")
                    nc.vector.tensor_scalar(
                        y[:], xt[:], nm[:], iv[:], op0=OP.add, op1=OP.mult)
                    sh_row = mod_vec_row(pool, mod_f, si, vsh, "shrow")
                    sc_row = mod_vec_row(pool, mod_f, si, vsc, "scrow")
                    for dt in range(6):
                        sl = slice(512 * dt, 512 * (dt + 1))
                        scp = bcast(psp, sc_row[:, sl], n, 512, "bc")
                        shp = bcast(psp, sh_row[:, sl], n, 512, "bc")
                        xm = pool.tile([n, 512], F32, name="xmsl")
                        nc.vector.tensor_mul(xm[:], y[:, sl], scp)
                        nc.vector.tensor_add(xm[:], xm[:], shp)
                        for j in range(4):
                            k = 4 * dt + j
                            tp = psp.tile([128, 128], F32, name="tp")[:, :n]
                            nc.tensor.transpose(
                                tp, xm[:, 128 * j:128 * (j + 1)], ident[:n, :n])
                            nc.vector.tensor_copy(xmT[:, k, off:off + n], tp)
                nc.sync.dma_start(
                    bounce.ap().rearrange("(k q) r -> q k r", q=128), xmT[:])

            def xmT_rhs(xmf, tbuf, k, t0, tw):
                """DMA a [128, tw] f32r rhs tile (contraction chunk k,
                tokens t0:t0+tw) out of the AllGathered xm^T buffer."""
                if t0 < LT:
                    r0, nr = t0 // TR, tw // TR
                    src = xmf.ap()[r0:r0 + nr, 128 * k:128 * (k + 1), 0:TR]
                else:
                    i0 = t0 - LT
                    r0, nr = i0 // IR, tw // IR
                    src = xmf.ap()[r0:r0 + nr, 128 * k:128 * (k + 1), TR:RR]
                nc.sync.dma_start(
                    tbuf.rearrange("q (r c) -> q r c", r=nr),
                    src.rearrange("r q c -> q r c").bitcast(F32R))

            def xmT_rhs_allk(xmf, tbuf, t0, tw):
                """[128, KD, tw] f32r rhs tile for all contraction chunks.
                One DMA per source rank (4-dim APs don't balance)."""
                if t0 < LT:
                    r0, nr, cw, c0 = t0 // TR, tw // TR, TR, 0
                else:
                    r0, nr, cw, c0 = (t0 - LT) // IR, tw // IR, IR, TR
                for j in range(nr):
                    src = xmf.ap()[r0 + j, :, c0:c0 + cw]
                    nc.sync.dma_start(
                        tbuf[:, :, cw * j:cw * (j + 1)],
                        src.rearrange("(k q) c -> q k c", q=128).bitcast(F32R))

            def part_out_dma(bounce, tc0, d0, src_tile):
                """[128, 512] partial tile (token chunk tc0) -> rank-blocked
                partial buffer rows."""
                t0 = 128 * tc0
                if t0 < LT:
                    for rr in range(4):
                        r = t0 // TR + rr
                        nc.sync.dma_start(
                            bounce.ap()[r, 0:TR, d0:d0 + 512],
                            src_tile[32 * rr:32 * (rr + 1), :])
                else:
                    r = (t0 - LT) // IR
                    nc.sync.dma_start(
                        bounce.ap()[r, TR:RR, d0:d0 + 512], src_tile)

            def part_out_dma_full(bounce, tc0, src_tile):
                """[128, D] partial rows (token chunk tc0) -> rank-blocked
                partial buffer."""
                t0 = 128 * tc0
                if t0 < LT:
                    for rr in range(4):
                        r = t0 // TR + rr
                        nc.sync.dma_start(
                            bounce.ap()[r, 0:TR, :],
                            src_tile[32 * rr:32 * (rr + 1), :])
                else:
                    r = (t0 - LT) // IR
                    nc.sync.dma_start(bounce.ap()[r, TR:RR, :], src_tile)

            # =============================================================
            # P0: silu(vec)
            vec_t = const.tile([128, KD], F32)
            nc.sync.dma_start(vec_t[:], p["vec"].ap().rearrange("(k q) -> q k", q=128))
            vec_s = const.tile([128, KD], F32R)
            nc.scalar.activation(vec_s[:], vec_t[:], AF.Silu)

            # x rows (persist: needed for residual in P5)
            x_t = {}
            for s in streams:
                off, n = ROWR[s]
                x_t[s] = xin.tile([n, D], F32, name=f"x_{s}")
                nc.sync.dma_start(x_t[s][:], p["x_rows"].ap()[off:off + n, :])
            x2_t = {}
            for s in streams:
                off, n = ROWR[s]
                x2_t[s] = persist.tile([n, D], F32, name=f"x2_{s}")

            # =============================================================
            with tc.tile_pool(name="modw", bufs=1) as modp, \
                 tc.tile_pool(name="modsb", bufs=1) as modsb, \
                 tc.tile_pool(name="modps", bufs=2, space="PSUM") as modps:

                def matvec(wparam, n_out, out_slice):
                    wview = wparam.ap().rearrange("(ko ki) n -> ki ko n", ki=128)
                    for nt in range(n_out // 384):
                        mp = modps.tile([1, 384], F32, name="mvps")
                        wt = modp.tile([128, KD, 384], F32R, name="mv_w")
                        nc.sync.dma_start(
                            wt[:],
                            wview[:, :, 384 * nt:384 * (nt + 1)].bitcast(F32R))
                        for k in range(KD):
                            nc.tensor.matmul(
                                mp[:], vec_s[:, k:k + 1], wt[:, k, :],
                                start=(k == 0), stop=(k == KD - 1))
                        nc.vector.tensor_copy(
                            out_slice[:, 384 * nt:384 * (nt + 1)], mp[:])

                # P1A: shift1/scale1
                gA = modsb.tile([1, 2, 768], F32, name="gA")
                for si, s in enumerate(streams):
                    matvec(p[f"mwA_{s}"], 768, gA[:, si, :])
                    bA = modp.tile([1, 768], F32, name="mbA")
                    nc.sync.dma_start(bA[:], p[f"mbA_{s}"].ap())
                    nc.vector.tensor_add(gA[:, si, :], gA[:, si, :], bA[:])
                    nc.vector.tensor_scalar_add(
                        gA[:, si, 384:768], gA[:, si, 384:768], 1.0)
                    nc.sync.dma_start(modA_b.ap()[si:si + 1, :], gA[0:1, si, :])
                nc.gpsimd.collective_compute(
                    "AllGather", OP.bypass, replica_groups=RG,
                    ins=[modA_b.ap().opt()], outs=[modA_f.ap().opt()])

                # P2: ln1 + modulate -> xm^T -> AllGather
                with tc.tile_pool(name="rw2", bufs=1) as rw2, \
                     tc.tile_pool(name="rps2", bufs=1, space="PSUM") as _rps2, \
                     tc.tile_pool(name="rps2b", bufs=3, space="PSUM") as rps2:
                    ln_mod_transpose(rw2, rps2, x_t, modA_f, 0, 1, xm_b)
                nc.gpsimd.collective_compute(
                    "AllGather", OP.bypass, replica_groups=RG,
                    ins=[xm_b.ap().opt()], outs=[xm_f.ap().opt()])

                # P1B: gate1/shift2/scale2/gate2 (traced after the critical
                # P2 chain so its DMAs don't get priority over it)
                gB = modsb.tile([1, 2, 1536], F32, name="gB")
                for si, s in enumerate(streams):
                    matvec(p[f"mwB_{s}"], 1536, gB[:, si, :])
                    bB = modp.tile([1, 1536], F32, name="mbB")
                    nc.sync.dma_start(bB[:], p[f"mbB_{s}"].ap())
                    nc.vector.tensor_add(gB[:, si, :], gB[:, si, :], bB[:])
                    nc.vector.tensor_scalar_add(
                        gB[:, si, 768:1152], gB[:, si, 768:1152], 1.0)
                    nc.sync.dma_start(modB_b.ap()[si:si + 1, :], gB[0:1, si, :])
                nc.gpsimd.collective_compute(
                    "AllGather", OP.bypass, replica_groups=RG,
                    ins=[modB_b.ap().opt()], outs=[modB_f.ap().opt()])

            # =============================================================
            # P3/P4: qkv, rope, attention, proj
            with tc.tile_pool(name="qk", bufs=1) as qk:
                qkT = qk.tile([128, 6, L], F32R, name="qkT")
                vT = qk.tile([128, 3, L], F32, name="vT")
                attnT = qk.tile([128, HL, L], F32R, name="attnT")

                with tc.tile_pool(name="qkw", bufs=2) as qkp, \
                     tc.tile_pool(name="qkrhs", bufs=1) as qkrhs, \
                     tc.tile_pool(name="qkwk", bufs=1) as qwk, \
                     tc.tile_pool(name="qke", bufs=2) as qke:

                    qkv_psum = tc.tile_pool(name="qkps", bufs=3, space="PSUM")
                    qkps = qkv_psum.__enter__()
                    STOK = {"txt": [(0, 256)],
                            "img": [(256, 256), (512, 256),
                                    (768, 256), (1024, 256)]}
                    for s in streams:
                        for (t0, tw) in STOK[s]:
                            rhs_all = qkrhs.tile([128, KD, 256], F32R,
                                                 name="qk_rhs")
                            xmT_rhs_allk(xm_f, rhs_all[:, :, :tw], t0, tw)
                            wv = p[f"qkvw_{s}"].ap().rearrange(
                                "(ko ki) c -> ki ko c", ki=128)
                            for cb in range(9):
                                mp = qkps.tile([128, 512], F32,
                                               name="qk_ps")[:, :tw]
                                wt = qkp.tile([128, KD, 128], F32R, name="qk_w")
                                nc.sync.dma_start(
                                    wt[:],
                                    wv[:, :, 128 * cb:128 * (cb + 1)].bitcast(F32R))
                                for k in range(KD):
                                    nc.tensor.matmul(
                                        mp, wt[:, k, :], rhs_all[:, k, :tw],
                                        start=(k == 0), stop=(k == KD - 1))
                                dst = (qkT[:, cb, t0:t0 + tw] if cb < 6
                                       else vT[:, cb - 6, t0:t0 + tw])
                                nc.scalar.activation(
                                    dst, mp, AF.Identity,
                                    bias=qkvb_t[s][:, cb:cb + 1])

                    qkv_psum.__exit__(None, None, None)
                    rope_psum1 = tc.tile_pool(name="rops", bufs=1, space="PSUM")
                    rops = rope_psum1.__enter__()
                    rope_psum2 = tc.tile_pool(name="rops2", bufs=2, space="PSUM")
                    rops2 = rope_psum2.__enter__()
                    # rope + rms (in place on qkT)
                    for ci in range(6):
                        w = "qn" if ci < 3 else "kn"
                        src = qkT[:, ci, :]
                        sq = qwk.tile([128, L], F32R, name="rsq")
                        nc.vector.tensor_mul(sq[:], src, src)
                        ssp = rops.tile([1, L], F32, name="ssps")
                        for (t0, tw) in QT_TILES:
                            nc.tensor.matmul(
                                ssp[:, t0:t0 + tw], ones_c[:], sq[:, t0:t0 + tw],
                                start=True, stop=True)
                        inv = qwk.tile([1, L], F32R, name="rinv")
                        with nc.allow_low_precision(reason="f32r is full width"):
                            nc.scalar.activation(inv[:], ssp[:], AF.Sqrt,
                                                 bias=eps_t[:1], scale=1.0 / DH)
                            nc.vector.reciprocal(inv[:], inv[:])
                        xs = qwk.tile([128, L], F32, name="rxs")
                        nc.vector.stream_shuffle(xs[:], src, SWAP_MASK)
                        m1 = qwk.tile([128, L], F32, name="rm1")
                        nc.vector.tensor_mul(m1[:], src, peA[:])
                        nc.vector.tensor_mul(xs[:], xs[:], peB[:])
                        nc.vector.tensor_add(m1[:], m1[:], xs[:])
                        for (t0, tw, s) in SRANGES:
                            ip = bcast(rops2, inv[:, t0:t0 + tw], 128, tw, "rip")
                            nc.vector.scalar_tensor_tensor(
                                qkT[:, ci, t0:t0 + tw], m1[:, t0:t0 + tw],
                                wn_t[(w, s)][:], ip, op0=OP.mult, op1=OP.mult)

                    rope_psum2.__exit__(None, None, None)
                    rope_psum1.__exit__(None, None, None)
                    # attention per head
                    with tc.tile_pool(name="atps", bufs=2, space="PSUM") as atps, \
                         tc.tile_pool(name="scps", bufs=3, space="PSUM") as scps:
                        for h in range(HL):
                            vn = qwk.tile([128, 10, 128], F32R, name="vn")
                            for c in range(10):
                                tp = scps.tile([128, 128], F32, name="sc")
                                nc.tensor.transpose(
                                    tp[:], vT[:, h, 128 * c:128 * (c + 1)],
                                    ident[:])
                                nc.vector.tensor_copy(vn[:, c, :], tp[:])
                            for (t0, tw) in QT_TILES:
                                sume = atps.tile([1, 512], F32,
                                                 name="sume")[:, :tw]
                                acc = atps.tile([128, 512], F32,
                                                name="acc")[:, :tw]
                                for c in range(10):
                                    sc = scps.tile([128, 512], F32,
                                                   name="sc")[:, :tw]
                                    nc.tensor.matmul(
                                        sc, qkT[:, 3 + h, 128 * c:128 * (c + 1)],
                                        qkT[:, h, t0:t0 + tw],
                                        start=True, stop=True)
                                    et = qke.tile([128, 512], F32R,
                                                  name="expt")[:, :tw]
                                    nc.scalar.activation(et, sc, AF.Exp,
                                                         scale=SQD)
                                    nc.tensor.matmul(sume, ones_c[:], et,
                                                     start=(c == 0),
                                                     stop=(c == 9))
                                    nc.tensor.matmul(acc, vn[:, c, :], et,
                                                     start=(c == 0),
                                                     stop=(c == 9))
                                rec = qke.tile([1, 512], F32R,
                                               name="rec")[:, :tw]
                                with nc.allow_low_precision(reason="f32r"):
                                    nc.vector.reciprocal(rec, sume)
                                rp = bcast(scps, rec, 128, tw, "sc")
                                acc_sb = qke.tile([128, 512], F32,
                                                  name="accsb")[:, :tw]
                                nc.scalar.copy(acc_sb, acc)
                                nc.vector.tensor_mul(
                                    attnT[:, h, t0:t0 + tw], acc_sb, rp)

                # proj partials
                with tc.tile_pool(name="prj", bufs=1) as prj, \
                     tc.tile_pool(name="prps", bufs=5, space="PSUM") as prps:
                    TGROUPS = [[0, 1], [2, 3, 4, 5, 6], [7, 8, 9]]
                    for tg in TGROUPS:
                        s = "txt" if tg[0] < 2 else "img"
                        pwt = prj.tile([128, HL, D], F32R, name="pw", bufs=1)
                        nc.sync.dma_start(
                            pwt[:],
                            p[f"pw_{s}"].ap().rearrange(
                                "(h q) c -> q h c", q=128).bitcast(F32R))
                        for t in tg:
                            evt = prj.tile([128, D], F32, name="pev", bufs=3)
                            for d in range(6):
                                pst = prps.tile([128, 512], F32, name="pt")
                                for hh in range(HL):
                                    nc.tensor.matmul(
                                        pst,
                                        attnT[:, hh, 128 * t:128 * (t + 1)],
                                        pwt[:, hh, 512 * d:512 * (d + 1)],
                                        start=(hh == 0), stop=(hh == HL - 1))
                                nc.scalar.copy(evt[:, 512 * d:512 * (d + 1)], pst)
                            part_out_dma_full(pp_b, t, evt[:])
            nc.gpsimd.collective_compute(
                "ReduceScatter", OP.add, replica_groups=RG,
                ins=[pp_b.ap().opt()], outs=[pp_r.ap().opt()])

            # =============================================================
            # P5: residual + gate; ln2 + modulate -> xm2^T -> AllGather
            with tc.tile_pool(name="rw5", bufs=1) as rw5, \
                 tc.tile_pool(name="rps5", bufs=1, space="PSUM") as _r5, \
                 tc.tile_pool(name="rps5b", bufs=3, space="PSUM") as rps5:
                for si, s in enumerate(streams):
                    off, n = ROWR[s]
                    prt = rw5.tile([n, D], F32, name=f"pr_{s}")
                    nc.sync.dma_start(prt[:], pp_r.ap()[off:off + n, :])
                    pbr = rw5.tile([1, D], F32R, name="pbr")
                    nc.sync.dma_start(pbr[:], p[f"pb_{s}"].ap().bitcast(F32R))
                    g1r = mod_vec_row(rw5, modB_f, si, 0, "g1row")
                    x2 = x2_t[s]
                    for dt in range(6):
                        sl = slice(512 * dt, 512 * (dt + 1))
                        pbp = bcast(rps5, pbr[:, sl], n, 512, "bc")
                        g1p = bcast(rps5, g1r[:, sl], n, 512, "bc")
                        nc.vector.tensor_add(x2[:, sl], prt[:, sl], pbp)
                        nc.vector.tensor_mul(x2[:, sl], x2[:, sl], g1p)
                        nc.vector.tensor_add(x2[:, sl], x2[:, sl], x_t[s][:, sl])
                ln_mod_transpose(rw5, rps5, x2_t, modB_f, 1, 2, xm2_b)
            nc.gpsimd.collective_compute(
                "AllGather", OP.bypass, replica_groups=RG,
                ins=[xm2_b.ap().opt()], outs=[xm2_f.ap().opt()])
            xin_pool.__exit__(None, None, None)

            # =============================================================
            # P6/P7: mlp per token-half
            HALF = [
                (0, [(0, 256, "txt"), (256, 384, "img")], [0, 1, 2, 3, 4]),
                (640, [(640, 320, "img"), (960, 320, "img")], [5, 6, 7, 8, 9]),
            ]
            with tc.tile_pool(name="mlrhs", bufs=1) as mlrhs, \
                 tc.tile_pool(name="mlw", bufs=1) as mlw:
                for (h0, ranges, tchunks) in HALF:
                    rhs_half = mlrhs.tile([128, KD, 640], F32R, name="ml_rhs")
                    for t in tchunks:
                        o = 128 * t - h0
                        xmT_rhs_allk(xm2_f, rhs_half[:, :, o:o + 128],
                                     128 * t, 128)
                    hid = mlrhs.tile([128, KML, 640], F32R, name="ml_hid")
                    with tc.tile_pool(name="mlps", bufs=3,
                                      space="PSUM") as mlps:
                        hstreams = sorted(set(x[2] for x in ranges))
                        for hc in range(KML):
                            pst = {}
                            for (t0, tw, s) in ranges:
                                pst[t0] = mlps.tile([128, 512], F32,
                                                    name="mlp")[:, :tw]
                            w1t = {}
                            for s in hstreams:
                                w1t[s] = mlw.tile([128, KD, 128], F32R,
                                                  name="w1", bufs=2)
                                nc.sync.dma_start(
                                    w1t[s][:],
                                    p[f"w1_{s}"].ap().rearrange(
                                        "(ko ki) c -> ki ko c", ki=128)
                                    [:, :, 128 * hc:128 * (hc + 1)]
                                    .bitcast(F32R))
                            for k in range(KD):
                                for (t0, tw, s) in ranges:
                                    nc.tensor.matmul(
                                        pst[t0], w1t[s][:, k, :],
                                        rhs_half[:, k, t0 - h0:t0 - h0 + tw],
                                        start=(k == 0), stop=(k == KD - 1))
                            for (t0, tw, s) in ranges:
                                nc.scalar.activation(
                                    hid[:, hc, t0 - h0:t0 - h0 + tw], pst[t0],
                                    AF.Gelu_apprx_tanh,
                                    bias=b1_t[s][:, hc:hc + 1])
                    with tc.tile_pool(name="m2ps", bufs=5,
                                      space="PSUM") as m2ps:
                        for d in range(6):
                            pst2 = {}
                            for t in tchunks:
                                pst2[t] = m2ps.tile([128, 512], F32, name="m2")
                            w2t = {}
                            for s in hstreams:
                                w2t[s] = mlw.tile([128, KML, 512], F32R,
                                                  name="w2", bufs=2)
                                nc.sync.dma_start(
                                    w2t[s][:],
                                    p[f"w2_{s}"].ap().rearrange(
                                        "(ko ki) c -> ki ko c", ki=128)
                                    [:, :, 512 * d:512 * (d + 1)]
                                    .bitcast(F32R))
                            for hc in range(KML):
                                for t in tchunks:
                                    s = "txt" if t < 2 else "img"
                                    nc.tensor.matmul(
                                        pst2[t],
                                        hid[:, hc,
                                            128 * t - h0:128 * (t + 1) - h0],
                                        w2t[s][:, hc, :], start=(hc == 0),
                                        stop=(hc == KML - 1))
                            for t in tchunks:
                                ev = mlw.tile([128, 512], F32, name="m2ev",
                                              bufs=3)
                                nc.scalar.copy(ev[:], pst2[t])
                                part_out_dma(mp_b, t, 512 * d, ev[:])
            nc.gpsimd.collective_compute(
                "ReduceScatter", OP.add, replica_groups=RG,
                ins=[mp_b.ap().opt()], outs=[mp_r.ap().opt()])

            # =============================================================
            # P8: final residual + gate -> out_rows
            with tc.tile_pool(name="rw8", bufs=1) as rw8, \
                 tc.tile_pool(name="rps8", bufs=3, space="PSUM") as rps8:
                for si, s in enumerate(streams):
                    off, n = ROWR[s]
                    mrt = rw8.tile([n, D], F32, name=f"mr_{s}")
                    nc.sync.dma_start(mrt[:], mp_r.ap()[off:off + n, :])
                    b2r = rw8.tile([1, D], F32R, name="b2r")
                    nc.sync.dma_start(b2r[:], p[f"b2_{s}"].ap().bitcast(F32R))
                    g2r = mod_vec_row(rw8, modB_f, si, 3, "g2row")
                    fin = rw8.tile([n, D], F32, name=f"fin_{s}")
                    for dt in range(6):
                        sl = slice(512 * dt, 512 * (dt + 1))
                        b2p = bcast(rps8, b2r[:, sl], n, 512, "bc")
                        g2p = bcast(rps8, g2r[:, sl], n, 512, "bc")
                        nc.vector.tensor_add(fin[:, sl], mrt[:, sl], b2p)
                        nc.vector.tensor_mul(fin[:, sl], fin[:, sl], g2p)
                        nc.vector.tensor_add(fin[:, sl], fin[:, sl],
                                             x2_t[s][:, sl])
                    nc.sync.dma_start(out_rows.ap()[off:off + n, :], fin[:])

    _split_fat_waits(nc)
    return nc


# ---------------------------------------------------------------------------
# host-side sharding / execution
_CACHE = {}


def _host_shards(inputs):
    f = lambda k: np.ascontiguousarray(np.asarray(inputs[k], np.float32))
    img, txt, vec, pe = f("img")[0], f("txt")[0], f("vec")[0], f("pe")
    pe4 = pe[0, 0]  # [L, 64, 2, 2]
    A = np.empty((L, DH), np.float32)
    B = np.empty((L, DH), np.float32)
    A[:, 0::2] = pe4[:, :, 0, 0]; A[:, 1::2] = pe4[:, :, 1, 1]
    B[:, 0::2] = pe4[:, :, 0, 1]; B[:, 1::2] = pe4[:, :, 1, 0]
    pe_a = np.ascontiguousarray(A.T)
    pe_b = np.ascontiguousarray(B.T)

    maps = []
    for r in range(NCORES):
        m = {
            "vec": vec, "pe_a": pe_a, "pe_b": pe_b,
            "ones_row": np.ones((1, 128), np.float32),
            "ones_col": np.ones((128, 1), np.float32),
            "x_rows": np.concatenate(
                [txt[TR * r:TR * (r + 1)], img[IR * r:IR * (r + 1)]], 0),
        }
        for s in ("img", "txt"):
            mw, mb = f(f"{s}_mod_w"), f(f"{s}_mod_b")
            cA = np.concatenate([np.arange(D * v + 384 * r, D * v + 384 * (r + 1))
                                 for v in (0, 1)])
            cB = np.concatenate([np.arange(D * v + 384 * r, D * v + 384 * (r + 1))
                                 for v in (2, 3, 4, 5)])
            m[f"mwA_{s}"] = np.ascontiguousarray(mw[:, cA])
            m[f"mbA_{s}"] = np.ascontiguousarray(mb[cA][None])
            m[f"mwB_{s}"] = np.ascontiguousarray(mw[:, cB])
            m[f"mbB_{s}"] = np.ascontiguousarray(mb[cB][None])
            qw, qb = f(f"{s}_qkv_w"), f(f"{s}_qkv_b")
            hs = [HL * r + j for j in range(HL)]
            cols = np.concatenate(
                [np.arange(D * blk + DH * h, D * blk + DH * (h + 1))
                 for blk in range(3) for h in hs])
            m[f"qkvw_{s}"] = np.ascontiguousarray(qw[:, cols])
            m[f"qkvb_{s}"] = np.ascontiguousarray(qb[cols])
            m[f"qn_{s}"] = f(f"{s}_qnorm")
            m[f"kn_{s}"] = f(f"{s}_knorm")
            rowsel = np.concatenate(
                [np.arange(DH * h, DH * (h + 1)) for h in hs])
            m[f"pw_{s}"] = np.ascontiguousarray(f(f"{s}_proj_w")[rowsel])
            m[f"pb_{s}"] = f(f"{s}_proj_b")[None]
            m[f"w1_{s}"] = np.ascontiguousarray(
                f(f"{s}_mlp_w1")[:, ML * r:ML * (r + 1)])
            m[f"b1_{s}"] = np.ascontiguousarray(
                f(f"{s}_mlp_b1")[ML * r:ML * (r + 1)])
            m[f"w2_{s}"] = np.ascontiguousarray(
                f(f"{s}_mlp_w2")[ML * r:ML * (r + 1)])
            m[f"b2_{s}"] = f(f"{s}_mlp_b2")[None]
        maps.append(m)
    return maps


def _build_exec(nc, n_cores):
    import jax
    from jax.sharding import Mesh, PartitionSpec
    from jax.experimental.shard_map import shard_map
    from concourse.bass2jax import (
        _bass_exec_p, install_neuronx_cc_hook, partition_id_tensor)

    install_neuronx_cc_hook()
    partition_name = (nc.partition_id_tensor.name
                      if nc.partition_id_tensor else None)
    in_names, out_names, out_avals, zero_outs = [], [], [], []
    for alloc in nc.m.functions[0].allocations:
        if not isinstance(alloc, mybir.MemoryLocationSet):
            continue
        name = alloc.memorylocations[0].name
        if alloc.kind == "ExternalInput":
            if name != partition_name:
                in_names.append(name)
        elif alloc.kind == "ExternalOutput":
            shape = tuple(alloc.tensor_shape)
            dtype = mybir.dt.np(alloc.dtype)
            out_names.append(name)
            out_avals.append(jax.core.ShapedArray(shape, dtype))
            zero_outs.append(np.zeros(shape, dtype))
    n_params, n_outs = len(in_names), len(out_avals)
    all_in = list(in_names) + list(out_names)
    if partition_name is not None:
        all_in.append(partition_name)

    def _body(*args):
        operands = list(args)
        if partition_name is not None:
            operands.append(partition_id_tensor())
        return tuple(_bass_exec_p.bind(
            *operands, out_avals=tuple(out_avals), in_names=tuple(all_in),
            out_names=tuple(out_names), lowering_input_output_aliases=(),
            sim_require_finite=True, sim_require_nnan=True, nc=nc))

    devices = jax.devices()[:n_cores]
    mesh = Mesh(np.asarray(devices), ("core",))
    donate = tuple(range(n_params, n_params + n_outs))
    sharded = jax.jit(
        shard_map(_body, mesh=mesh,
                  in_specs=(PartitionSpec("core"),) * (n_params + n_outs),
                  out_specs=(PartitionSpec("core"),) * n_outs,
                  check_rep=False),
        donate_argnums=donate, keep_unused=True)
    return sharded, in_names, out_names, out_avals, zero_outs, mesh


def _get_compiled():
    if "exec" not in _CACHE:
        nc = build_nc()
        _CACHE["exec"] = _build_exec(nc, NCORES)
    return _CACHE["exec"]


def run_sharded(in_maps, n_timing=0):
    import jax
    from jax.sharding import PartitionSpec, NamedSharding
    sharded, in_names, out_names, out_avals, zero_outs, mesh = _get_compiled()
    sharding = NamedSharding(mesh, PartitionSpec("core"))
    concat_in = []
    for k in in_names:
        a0 = np.asarray(in_maps[0][k])
        cat = np.concatenate([np.asarray(in_maps[c][k]) for c in range(NCORES)], 0)
        concat_in.append(np.ascontiguousarray(cat))
    dev_in = [jax.device_put(a, sharding) for a in concat_in]
    for a in dev_in:
        a.block_until_ready()

    def zeros():
        return [jax.device_put(
            np.zeros((NCORES * z.shape[0], *z.shape[1:]), z.dtype), sharding)
            for z in zero_outs]

    outs = sharded(*dev_in, *zeros())
    jax.block_until_ready(outs)
    results = [
        {name: np.asarray(outs[i]).reshape(NCORES, *out_avals[i].shape)[c]
         for i, name in enumerate(out_names)}
        for c in range(NCORES)]
    times = []
    for _ in range(n_timing):
        zs = zeros()
        jax.block_until_ready(zs)
        t0 = time.perf_counter()
        o = sharded(*dev_in, *zs)
        jax.block_until_ready(o)
        times.append(time.perf_counter() - t0)
    return results, times


def kernel(**inputs):
    maps = _host_shards(inputs)
    results, _ = run_sharded(maps, n_timing=0)
    img = np.empty((1, LI, D), np.float32)
    txt = np.empty((1, LT, D), np.float32)
    for r in range(NCORES):
        rows = results[r]["out_rows"]
        txt[0, TR * r:TR * (r + 1)] = rows[:TR]
        img[0, IR * r:IR * (r + 1)] = rows[TR:]
    return img, txt
